# revision 25
# baseline (speedup 1.0000x reference)
"""GQA causal-attention prefill kernel for 8 Trainium2 NeuronCores.

Reference computation (B=2, S=2048, D=4096, Q=32 q-heads, N=8 kv-heads,
H=128): QKV projection + RoPE + causal GQA attention + O projection.

Sharding: core c handles batch b = c//4 and kv-head pair g = c%4
(kv-heads 2g..2g+1, q-heads 8g..8g+7).  No collectives: each core
computes its partial o-projection (sum over its 8 q-heads) and the host
sums the four partials per batch at gather time (the "all-reduce").

Device-side strategy (v2, bf16 pipeline):
  - all matmul operands are bf16 (PSUM accumulation stays fp32); host
    pre-casts x and the weights.  Measured end-to-end error ~4e-3
    rel-to-absmax vs the 2e-2 gate.
  - x is fed pre-transposed ([D, S]); weight DRAM layouts are host-
    permuted so every DMA line is contiguous per partition.
  - scores are computed transposed (psum [t, s]) so softmax weights are
    produced in the layout the AV contraction needs.
  - AV is flipped: stationary = 128-column chunks of the weight tile,
    moving = V with a 129th all-ones column, so the softmax denominator
    accumulates for free as psum column 128 (no separate den matmuls).
  - normalization is fused into PSUM evacuation: reciprocal of the den
    column feeds a per-partition scale on the ScalarE copy.
  - attention output is transposed back to [h, s] on TensorE (bf16
    identity transpose) and kept in SBUF for the o-projection; no DRAM
    intermediates anywhere.
  - diagonal score tiles are column-restricted (causal trimming) and
    fully-masked AV chunks are skipped.
  - q-projection of tile j+1 is emitted before attention of tile j so
    the RoPE chain latency never stalls the PE.
"""

import math
import sys

import numpy as np

for _p in ("/opt/trn_rl_repo", "/root/.axon_site/_ro/trn_rl_repo"):
    if _p not in sys.path:
        sys.path.append(_p)

import ml_dtypes

import concourse.bacc as bacc
import concourse.mybir as mybir
import concourse.tile as tile
from concourse import bass_utils

dt = mybir.dt
F32 = dt.float32
F32R = dt.float32r
BF16 = dt.bfloat16
ADD = mybir.AluOpType.add
MULT = mybir.AluOpType.mult
EXP = mybir.ActivationFunctionType.Exp
COPY = mybir.ActivationFunctionType.Copy

FULL_CFG = dict(S=2048, D=4096, QH=8, KH=2, H=128, SC=512, SCQ=512, ST=512)
N_CORES = 8
ROPE_THETA = 10000.0
NEG_BIG = -1.0e30
AV_LAG = 2  # kt-lag between the scores matmul and the AV consumption


def build_bass(cfg):
    S, D, QH, KH, H = cfg["S"], cfg["D"], cfg["QH"], cfg["KH"], cfg["H"]
    SC, SCQ, ST = cfg["SC"], cfg["SCQ"], cfg["ST"]
    HG = 4                 # q-heads per weight-resident group
    assert H == 128 and D % 128 == 0 and S % SC == 0
    DT = D // 128          # d-tiles (contraction tiles for projections)
    NCH = S // SC          # x chunks in pass A
    NJ = S // ST           # s-tiles for attention
    TJ = ST // 128         # 128-wide t-tiles per attention s-tile
    NT = S // 128          # total t-tiles
    G = QH // KH           # GQA group size
    EW = 512               # o-proj output tile width
    NE = D // EW
    VW = 130               # v1 row stride (129 used + 1 pad for alignment)
    scale = 1.0 / math.sqrt(H)

    from contextlib import ExitStack

    nc = bacc.Bacc("TRN2", target_bir_lowering=False, debug=False,
                   enable_asserts=False, num_devices=N_CORES)

    xT = nc.dram_tensor("xT", [D, S], BF16, kind="ExternalInput")
    wq = nc.dram_tensor("wq", [QH, 128, DT, H], BF16, kind="ExternalInput")
    wk = nc.dram_tensor("wk", [KH, 128, DT, H], BF16, kind="ExternalInput")
    wv = nc.dram_tensor("wv", [KH, 128, DT, H], BF16, kind="ExternalInput")
    wo = nc.dram_tensor("wo", [QH, H, D], BF16, kind="ExternalInput")
    cos_d = nc.dram_tensor("cos_t", [128, S], BF16, kind="ExternalInput")
    sin_d = nc.dram_tensor("sin_t", [128, S], BF16, kind="ExternalInput")
    tri_d = nc.dram_tensor("tri_t", [128, 128], F32, kind="ExternalInput")
    idn_d = nc.dram_tensor("idn_t", [128, 128], BF16, kind="ExternalInput")
    perm_d = nc.dram_tensor("perm_t", [128, 128], BF16, kind="ExternalInput")
    o_out = nc.dram_tensor("o_out", [S, D], BF16, kind="ExternalOutput")

    xT_r = xT.ap().rearrange("(dt p) s -> p dt s", p=128)

    with tile.TileContext(nc) as tc, \
         nc.allow_low_precision(reason="deliberate bf16 pipeline"):
        with tc.tile_pool(name="persist", bufs=1) as persist:
            cos_sb = persist.tile([128, S], BF16)
            sin_sb = persist.tile([128, S], BF16)
            tri_sb = persist.tile([128, 128], F32)
            idn_sb = persist.tile([128, 128], BF16)
            perm_sb = persist.tile([128, 128], BF16)
            k_sb = persist.tile([128, KH, S], BF16)
            v1_sb = persist.tile([128, NT, KH, VW], BF16)
            otT_sb = persist.tile([128, QH, S], BF16)

            def load_tables():
                nc.scalar.dma_start(cos_sb[:], cos_d[:, :])
                nc.scalar.dma_start(sin_sb[:], sin_d[:, :])
                nc.scalar.dma_start(tri_sb[:], tri_d[:, :])
                nc.scalar.dma_start(idn_sb[:], idn_d[:, :])
                nc.scalar.dma_start(perm_sb[:], perm_d[:, :])

            def rope_mults(ps_tile, s0, W, rp):
                """Stage 1 of rope: evacuate the projection psum to bf16
                via ScalarE (freeing the psum bank quickly) and compute the
                cos/sin products on the otherwise-idle GpSimd engine."""
                qb = rp.tile([128, W], BF16, tag="qb")
                nc.scalar.copy(qb[:], ps_tile)
                ta = rp.tile([128, W], BF16, tag="ta")
                tb = rp.tile([128, W], BF16, tag="tb")
                nc.gpsimd.tensor_tensor(ta[:], qb[:], cos_sb[:, s0:s0 + W],
                                        MULT)
                nc.gpsimd.tensor_tensor(tb[:], qb[:], sin_sb[:, s0:s0 + W],
                                        MULT)
                return ta, tb

            def rope_fin(ta, tb, dst_ap, swp, swtag):
                """Stage 2: rotate-half on TensorE (constant permutation
                matmul, sign folded into the sin table) + final add."""
                W = ta.shape[1]
                tbs = swp.tile([128, W], F32, tag=swtag)
                nc.tensor.matmul(tbs[:], perm_sb[:], tb[:],
                                 start=True, stop=True)
                nc.vector.tensor_tensor(dst_ap, ta[:], tbs[:], ADD)

            # wq head tiles live in pools opened before pass A so the
            # hg=0 weight loads overlap the k/v projection.
            wqp_es = ExitStack()
            wqp = wqp_es.enter_context(tc.tile_pool(name="wqp", bufs=1))
            xtp_es = ExitStack()
            xtsp = xtp_es.enter_context(tc.tile_pool(name="xts", bufs=2))

            def load_wq_group(hg, dst):
                for hl in range(HG):
                    wt_ = wqp.tile([128, DT, H], BF16, tag="wq%d" % hl,
                                   name="wq_%d_%d" % (hg, hl))
                    nc.scalar.dma_start(wt_[:], wq.ap()[hg * HG + hl])
                    dst.append(wt_)

            # ---- PASS A: k and v projections (+ RoPE on k) ----
            wq_h0 = []
            with nc.named_scope("passA"), \
                 tc.tile_pool(name="wkv", bufs=1) as wkvp, \
                 tc.tile_pool(name="ropeA", bufs=2) as rpA, \
                 tc.tile_pool(name="pskA", bufs=2, space="PSUM") as psk, \
                 tc.tile_pool(name="psvA", bufs=3, space="PSUM") as psv, \
                 tc.tile_pool(name="pswA", bufs=2, space="PSUM") as psw:
                wk_t = wkvp.tile([128, KH, DT, H], BF16)
                wv_t = wkvp.tile([128, DT, KH, H], BF16)
                hdt = DT // 2
                # startup feed order matched to consumption: wk halves
                # interleave with the first x chunk's quarters on the scalar
                # queue; wv and the tables follow; wq waits until the end.
                nc.scalar.dma_start(wk_t[:, 0, 0:hdt], wk.ap()[0][:, 0:hdt])
                nc.scalar.dma_start(wk_t[:, 1, 0:hdt], wk.ap()[1][:, 0:hdt])
                xtiles = []
                qdt = DT // 4
                for ch in range(NCH):
                    xts = xtsp.tile([128, DT, SC], BF16, tag="xts",
                                    name="xtsA%d" % ch)
                    xtiles.append(xts)

                def load_x(ch):
                    sl = ch * SC
                    for qq in range(4):
                        eng = nc.sync if qq % 2 == 0 else nc.scalar
                        eng.dma_start(xtiles[ch][:, qq * qdt:(qq + 1) * qdt],
                                      xT_r[:, qq * qdt:(qq + 1) * qdt,
                                           sl:sl + SC])

                load_x(0)
                nc.scalar.dma_start(wk_t[:, 0, hdt:DT], wk.ap()[0][:, hdt:DT])
                nc.scalar.dma_start(wk_t[:, 1, hdt:DT], wk.ap()[1][:, hdt:DT])
                load_tables()
                nc.gpsimd.memset(
                    v1_sb[:, :, :, 128:129].rearrange(
                        "p a b c -> p (a b c)"), 1.0)
                load_x(1)
                for n in range(KH):
                    nc.scalar.dma_start(wv_t[:, :, n, :], wv.ap()[n])
                load_x(2)
                load_x(3)
                load_wq_group(0, wq_h0)

                def kblock(ch):
                    outs = []
                    for kh in range(KH):
                        pk = psk.tile([128, SC], F32, tag="pk")
                        for di in range(DT):
                            nc.tensor.matmul(
                                pk[:], wk_t[:, kh, di, :],
                                xtiles[ch][:, di, :],
                                start=(di == 0), stop=(di == DT - 1))
                        ta, tb = rope_mults(pk[:], ch * SC, SC, rpA)
                        outs.append(
                            (ta, tb, k_sb[:, kh, ch * SC:(ch + 1) * SC]))
                    return outs

                def vblock(ch):
                    for tl in range(SC // 128):
                        pv = psv.tile([128, KH * H], F32, tag="pv")
                        for di in range(DT):
                            nc.tensor.matmul(
                                pv[:],
                                xtiles[ch][:, di, tl * 128:(tl + 1) * 128],
                                wv_t[:, di].rearrange("p a b -> p (a b)"),
                                start=(di == 0), stop=(di == DT - 1))
                        tt = ch * (SC // 128) + tl
                        for kh in range(KH):
                            nc.vector.tensor_copy(
                                v1_sb[:, tt, kh, 0:128],
                                pv[:, kh * H:(kh + 1) * H])

                # K blocks lead (V's weights arrive later); each rope
                # finisher trails behind the following block's matmuls.
                pend = []
                for step in ("K0", "K1", "V0", "K2", "V1", "K3", "V2", "V3"):
                    kind, ch = step[0], int(step[1])
                    if kind == "K":
                        new = kblock(ch)
                    else:
                        vblock(ch)
                        new = []
                    while pend:
                        ta, tb, dst = pend.pop(0)
                        rope_fin(ta, tb, dst, psw, "tbs")
                    pend = new
                for ta, tb, dst in pend:
                    rope_fin(ta, tb, dst, psw, "tbs")

            # ---- FUSED PASS: q projection + RoPE + attention ----
            es = ExitStack()
            with es:
                es.enter_context(wqp_es.pop_all())
                es.enter_context(xtp_es.pop_all())
                rpB = es.enter_context(tc.tile_pool(name="ropeB", bufs=2))
                qjp = es.enter_context(tc.tile_pool(name="qj", bufs=8))
                wtp = es.enter_context(tc.tile_pool(name="wt", bufs=4))
                anp = es.enter_context(tc.tile_pool(name="an", bufs=2))
                rcpp = es.enter_context(tc.tile_pool(name="rcp", bufs=2))
                psq = es.enter_context(
                    tc.tile_pool(name="psqB", bufs=2, space="PSUM"))
                pss = es.enter_context(
                    tc.tile_pool(name="pss", bufs=2, space="PSUM"))
                pop = es.enter_context(
                    tc.tile_pool(name="po", bufs=2, space="PSUM"))
                psT = es.enter_context(
                    tc.tile_pool(name="psT", bufs=2, space="PSUM"))

                def new_xts(j):
                    xts = xtsp.tile([128, DT, ST], BF16, tag="xts")
                    hdt = DT // 2
                    s0 = j * ST
                    nc.sync.dma_start(xts[:, 0:hdt],
                                      xT_r[:, 0:hdt, s0:s0 + ST])
                    nc.scalar.dma_start(xts[:, hdt:DT],
                                        xT_r[:, hdt:DT, s0:s0 + ST])
                    return xts

                def qproj_mms(wq_t, xts, j):
                    pq = psq.tile([128, ST], F32, tag="pq")
                    for di in range(DT):
                        nc.tensor.matmul(
                            pq[:], wq_t[:, di, :], xts[:, di, :],
                            start=(di == 0), stop=(di == DT - 1))
                    # the ScalarE evacuation + gpsimd multiplies free the pq
                    # psum buffer as soon as the projection completes ...
                    return rope_mults(pq[:], j * ST, ST, rpB)

                def qproj_rope(ta, tb, qj_hl):
                    # ... while the swap matmul + final add trail a stage
                    # behind, so the PE never waits on the multiplies.
                    rope_fin(ta, tb, qj_hl[:], psq, "pq")

                def attn_head(h, qjt, j):
                    kh = h // G
                    po = [pop.tile([128, 2, VW], F32, tag="po",
                                   name="po%d_%d_%d" % (h, j, i))
                          for i in range(2)]
                    KT = (j + 1) * TJ

                    def av(kt):
                        m = kt - j * TJ
                        wtile = wtiles[kt]
                        for c in range(max(m, 0), 4):
                            # start=True clears accumulate-bits for the WHOLE
                            # bank, so only the even chunk of each psum pair
                            # starts; the odd chunk's first matmul overwrites
                            # via the cleared bits.
                            nc.tensor.matmul(
                                po[c // 2][:, c % 2, 0:129],
                                wtile[:, c * 128:(c + 1) * 128],
                                v1_sb[:, kt, kh, 0:129],
                                start=(kt == 0 and c % 2 == 0),
                                stop=(kt == j * TJ + c),
                                skip_group_check=True)

                    wtiles = {}
                    for kt in range(KT):
                        m = kt - j * TJ
                        lo = max(m, 0) * 128
                        ps = pss.tile([128, ST], F32, tag="ps")
                        nc.tensor.matmul(
                            ps[:, lo:ST],
                            k_sb[:, kh, kt * 128:(kt + 1) * 128],
                            qjt[:, lo:ST], start=True, stop=True)
                        if m >= 0:
                            nc.vector.tensor_tensor(
                                ps[:, m * 128:(m + 1) * 128],
                                ps[:, m * 128:(m + 1) * 128],
                                tri_sb[:], ADD)
                        wtile = wtp.tile([128, ST], BF16, tag="wt")
                        nc.scalar.activation(wtile[:, lo:ST], ps[:, lo:ST],
                                             EXP, scale=scale)
                        wtiles[kt] = wtile
                        if kt >= AV_LAG:
                            av(kt - AV_LAG)
                    for kt in range(max(KT - AV_LAG, 0), KT):
                        av(kt)

                    # normalize + transpose back to [h, s] in SBUF
                    rcp = rcpp.tile([128, 4], F32, tag="rcp")
                    for i in range(2):
                        nc.vector.reciprocal(rcp[:, 2 * i:2 * i + 2],
                                             po[i][:, :, 128])
                    an = anp.tile([128, ST], BF16, tag="an")
                    for c in range(4):
                        nc.vector.tensor_scalar_mul(
                            an[:, c * 128:(c + 1) * 128],
                            po[c // 2][:, c % 2, 0:128],
                            rcp[:, c:c + 1])
                    for c in range(4):
                        pt = psT.tile([128, 128], BF16, tag="pt")
                        nc.tensor.transpose(
                            pt[:], an[:, c * 128:(c + 1) * 128], idn_sb[:])
                        nc.vector.tensor_copy(
                            otT_sb[:, h, j * ST + c * 128:j * ST + (c + 1) * 128],
                            pt[:])

                seq = [(hg, j) for hg in range(QH // HG) for j in range(NJ)]
                wq_groups = {0: wq_h0}

                def alloc_qj(hg, j):
                    return [qjp.tile([128, ST], BF16, tag="qj",
                                     name="qj%d_%d_%d" % (hg, j, hl))
                            for hl in range(HG)]

                # prologue: project the first s-tile for all 4 heads,
                # software-pipelined so each rope's swap matmul sits behind
                # the next head's projection matmuls.
                xts_cur = new_xts(0)
                qj_cur = alloc_qj(0, 0)
                tt_prev = None
                for hl in range(HG):
                    tt = qproj_mms(wq_groups[0][hl], xts_cur, 0)
                    if tt_prev is not None:
                        qproj_rope(*tt_prev, qj_cur[hl - 1])
                    tt_prev = tt
                qproj_rope(*tt_prev, qj_cur[HG - 1])

                pending = None  # (ta, tb, qj_tile) swap+add awaiting emission
                for idx, (hg, j) in enumerate(seq):
                    nxt = seq[idx + 1] if idx + 1 < len(seq) else None
                    if nxt is not None:
                        xts_nxt = new_xts(nxt[1])
                        qj_nxt = alloc_qj(*nxt)
                    elif pending is not None:
                        qproj_rope(*pending)
                        pending = None
                    for hl in range(HG):
                        # project head hl of the NEXT s-tile between
                        # attention heads of the current tile (ScalarE's exp
                        # stream never outruns the PE); each rope's swap
                        # matmul trails a full stage behind its projection.
                        if nxt is not None:
                            tt = qproj_mms(wq_groups[nxt[0]][hl], xts_nxt,
                                           nxt[1])
                            if pending is not None:
                                qproj_rope(*pending)
                            pending = (*tt, qj_nxt[hl])
                        attn_head(hg * HG + hl, qj_cur[hl], j)
                    if j == NJ - 2 and hg + 1 < QH // HG:
                        wq_groups[hg + 1] = []
                        load_wq_group(hg + 1, wq_groups[hg + 1])
                    if nxt is not None:
                        qj_cur = qj_nxt

            # ---- PASS C: o projection (partial over this core's heads) ----
            with nc.named_scope("passC"), \
                 tc.tile_pool(name="wop", bufs=2) as wop, \
                 tc.tile_pool(name="ocp", bufs=4) as ocp, \
                 tc.tile_pool(name="psc", bufs=4, space="PSUM") as psc:
                out_q = [nc.scalar, nc.gpsimd, nc.sync]
                for e in range(NE):
                    woe = wop.tile([128, QH, EW], BF16, tag="woe")
                    for h in range(QH):
                        nc.sync.dma_start(
                            woe[:, h, :], wo.ap()[h, :, e * EW:(e + 1) * EW])
                    for st in range(S // 128):
                        pc = psc.tile([128, EW], F32, tag="pc")
                        for h in range(QH):
                            nc.tensor.matmul(
                                pc[:],
                                otT_sb[:, h, st * 128:(st + 1) * 128],
                                woe[:, h, :],
                                start=(h == 0), stop=(h == QH - 1))
                        oc = ocp.tile([128, EW], BF16, tag="oc")
                        if st % 2 == 0:
                            nc.vector.tensor_copy(oc[:], pc[:])
                        else:
                            nc.scalar.copy(oc[:], pc[:])
                        out_q[st % 3].dma_start(
                            o_out[st * 128:(st + 1) * 128,
                                  e * EW:(e + 1) * EW], oc[:])

    nc.compile()
    return nc


def _perm_matrix():
    P = np.zeros((128, 128), dtype=np.float32)
    P[np.arange(128), (np.arange(128) + 64) % 128] = 1.0
    return P


def make_tables(positions_b, S, H):
    """cos/sin tables in [128, S] layout with the sign fold for the swap
    trick (rows 0:63 -> +sin, 64:127 -> -sin), plus the triangular mask."""
    half = H // 2
    inv_freq = 1.0 / (ROPE_THETA ** (np.arange(half, dtype=np.float64) * 2.0 / H))
    ang = positions_b.astype(np.float64)[None, :] * inv_freq[:, None]  # [half, S]
    cos_h = np.cos(ang)
    sin_h = np.sin(ang)
    cos_t = np.concatenate([cos_h, cos_h], axis=0).astype(ml_dtypes.bfloat16)
    sin_t = np.concatenate([sin_h, -sin_h], axis=0).astype(ml_dtypes.bfloat16)
    idx = np.arange(128)
    tri = np.where(idx[:, None] <= idx[None, :], 0.0, NEG_BIG).astype(np.float32)
    return cos_t, sin_t, tri


def _w_perm(w, DT):
    """[D, H] f32 -> [128, DT, H] bf16 (partition = d % 128)."""
    D, H = w.shape
    return np.ascontiguousarray(
        w.reshape(DT, 128, H).transpose(1, 0, 2)).astype(ml_dtypes.bfloat16)


def make_in_maps(x, positions, Wq, Wk, Wv, Wo, cfg):
    """Shard the full inputs into the 8 per-core input maps."""
    QH, KH = cfg["QH"], cfg["KH"]
    S, H, D = cfg["S"], cfg["H"], cfg["D"]
    DT = D // 128
    B = x.shape[0]
    groups = N_CORES // B
    tables = [make_tables(np.asarray(positions[b]), S, H) for b in range(B)]
    bf = ml_dtypes.bfloat16
    in_maps = []
    for c in range(N_CORES):
        b, g = divmod(c, groups)
        cos_t, sin_t, tri = tables[b]
        in_maps.append({
            "xT": np.ascontiguousarray(np.asarray(x[b]).T).astype(bf),
            "wq": np.stack([_w_perm(np.asarray(Wq[g * QH + h]), DT)
                            for h in range(QH)]),
            "wk": np.stack([_w_perm(np.asarray(Wk[g * KH + n]), DT)
                            for n in range(KH)]),
            "wv": np.stack([_w_perm(np.asarray(Wv[g * KH + n]), DT)
                            for n in range(KH)]),
            "wo": np.ascontiguousarray(Wo[g * QH:(g + 1) * QH]).astype(bf),
            "cos_t": cos_t,
            "sin_t": sin_t,
            "tri_t": tri,
            "idn_t": np.eye(128, dtype=np.float32).astype(bf),
            "perm_t": _perm_matrix().astype(bf),
        })
    return in_maps


_NC_CACHE = {}


def _get_nc(cfg_key=None):
    cfg = FULL_CFG if cfg_key is None else cfg_key
    key = tuple(sorted(cfg.items()))
    if key not in _NC_CACHE:
        _NC_CACHE[key] = build_bass(cfg)
    return _NC_CACHE[key]


def run(x, positions, Wq, Wk, Wv, Wo, trace=False, trace_kwargs=None):
    cfg = FULL_CFG
    nc = _get_nc(cfg)
    in_maps = make_in_maps(np.asarray(x), np.asarray(positions),
                           np.asarray(Wq), np.asarray(Wk), np.asarray(Wv),
                           np.asarray(Wo), cfg)
    res = bass_utils.run_bass_kernel_spmd(
        nc, in_maps, list(range(N_CORES)), trace=trace,
        **(trace_kwargs or {}))
    B = np.asarray(x).shape[0]
    groups = N_CORES // B
    outs = []
    for b in range(B):
        acc = res.results[b * groups]["o_out"].astype(np.float64)
        for g in range(1, groups):
            acc += res.results[b * groups + g]["o_out"]
        outs.append(acc.astype(np.float32))
    return np.stack(outs, axis=0), res


def kernel(x, positions, Wq, Wk, Wv, Wo):
    out, _ = run(x, positions, Wq, Wk, Wv, Wo, trace=False)
    return out
